# revision 1
# baseline (speedup 1.0000x reference)
"""NT-Xent style contrastive loss on 8 Trainium2 NeuronCores.

Math (matches the reference):
    z = l2norm_rows(concat([emb_i, emb_j]))            # [8192, 1024]
    sim = z @ z.T
    loss = mean_g( -(pos_g / t - log(sum_{j!=g} exp(sim[g,j]/t))) )
with t = 0.5, pos_g = sim[g, g^4096-ish pairing].

Because the final output is a scalar, only two reductions are needed:
    loss = ( sum_g log(denom_g) - (1/t) * sum_g pos_g ) / 8192

Distribution: each core is handed the full embedding matrix ROTATED so that
its 1024-row block sits at rows 0..1023.  All 8 cores then run an identical
(SPMD) program: compute the [1024 x 8192] block of sim, exp/row-reduce it,
and a 1024-wide slice of the positives.  Host sums the 8 partial pairs.

Per-core device pipeline:
  1. DMA row-major tiles [128, 1024] f32.
  2. ACT: cast to bf16.  DVE: fused square+row-sum -> norms2 [128,1].
  3. ACT: rnorm = exp(-0.5*ln(norms2))  (stays inside the exp/ln table set).
  4. PE: transpose+scale in one op:  psum = rows16[:, j*128:+128].T @ diag(rnorm)
     which lands z.T (normalized, transposed) chunks in PSUM; DVE copies them
     (cast bf16) into the resident ZT sbuf tensor [128, 8*8192] (k-tile major).
  5. PE: main matmul  sim_block = ZT[:, own_cols].T @ ZT  in [128,512] pieces
     accumulated over the 8 k-tiles into [128, 1024] PSUM windows.
  6. ACT: exp(2*x) in-place on PSUM with fused per-row accumulation
     (accum_out) -> rowsums.  denom = rowsums - e^2 (analytic self-term;
     |z|^2 = 1 to ~1e-4, the induced loss error is ~1e-7 relative).
  7. ACT ln -> PE ones-matmul partition reduction -> scalar partials.
  8. positives: pos[g] = z_g . z_{g+4096} = column-dot of ZT and its
     half-rotated self -> DVE elementwise mult + PE ones-matmul.
"""

import numpy as np
import ml_dtypes

N = 4096          # batch size (rows in emb_i / emb_j)
D = 1024          # embedding dim
R = 2 * N         # 8192 rows of z
BLK = R // 8      # 1024 rows per core
TEMP = 0.5
P = 128
KT = D // P       # 8 k-tiles
RT = R // P       # 64 row-tiles
E2 = float(np.exp(2.0))  # exp(sim_gg / t) with sim_gg == 1

_BF16 = ml_dtypes.bfloat16

_NC = None


def _build_nc(stages=("A", "B", "C", "D")):
    import concourse.bass as bass  # noqa: F401
    import concourse.tile as tile
    from concourse import bacc, mybir

    f32 = mybir.dt.float32
    bf16 = mybir.dt.bfloat16
    FT = mybir.ActivationFunctionType
    ALU = mybir.AluOpType

    nc = bacc.Bacc("TRN2", target_bir_lowering=False, debug=False, num_devices=8)

    emb = nc.dram_tensor("emb", [R, D], f32, kind="ExternalInput").ap()
    eye = nc.dram_tensor("eye128", [P, P], bf16, kind="ExternalInput").ap()
    onesb = nc.dram_tensor("ones_bf16", [P, 1], bf16, kind="ExternalInput").ap()
    onesf = nc.dram_tensor("ones_f32", [P, 1], f32, kind="ExternalInput").ap()
    outd = nc.dram_tensor("out", [1, 2], f32, kind="ExternalOutput").ap()

    with tile.TileContext(nc) as tc:
        with (
            tc.tile_pool(name="zt", bufs=1) as ztp,
            tc.tile_pool(name="io", bufs=6) as iop,
            tc.tile_pool(name="rows", bufs=4) as rowsp,
            tc.tile_pool(name="sq", bufs=2) as sqp,
            tc.tile_pool(name="small", bufs=4) as smallp,
            tc.tile_pool(name="diag", bufs=3) as diagp,
            tc.tile_pool(name="prod", bufs=2) as prodp,
            tc.tile_pool(name="stat", bufs=1) as statp,
            tc.tile_pool(name="ps", bufs=4, space="PSUM") as psp,
        ):
            # Resident normalized-transposed z, bf16.  k-tile k lives at
            # column offset k*R; global row r of z is column r of each k-tile.
            zt = ztp.tile([P, KT * R], bf16, tag="zt")

            eye_sb = statp.tile([P, P], bf16, tag="eye")
            nc.sync.dma_start(eye_sb[:], eye)
            ones_b = statp.tile([P, 1], bf16, tag="onesb")
            nc.sync.dma_start(ones_b[:], onesb)
            ones_f = statp.tile([P, 1], f32, tag="onesf")
            nc.sync.dma_start(ones_f[:], onesf)

            # 8 m-tiles x 8 n-windows of 1024
            rowsums = statp.tile([P, 64], f32, tag="rowsums")

            # ---------------- Phase A: normalize + transpose ----------------
            for rt in range(RT):
                raw = iop.tile([P, D], f32, tag="raw")
                nc.sync.dma_start(raw[:], emb[rt * P : (rt + 1) * P, :])

                r16 = rowsp.tile([P, D], bf16, tag="r16")
                nc.vector.tensor_copy(r16[:], raw[:])

                # norms2 via ACT Square with fused row-sum (square is present
                # in every ACT table set, so no table switch vs exp/ln).
                sq = sqp.tile([P, D], bf16, tag="sq")
                n2 = smallp.tile([P, 1], f32, tag="n2")
                nc.scalar.activation(sq[:], raw[:], FT.Square, accum_out=n2[:])

                lntmp = smallp.tile([P, 1], f32, tag="lntmp")
                nc.scalar.activation(lntmp[:], n2[:], FT.Ln)
                rn = smallp.tile([P, 1], f32, tag="rn")
                nc.scalar.activation(rn[:], lntmp[:], FT.Exp, scale=-0.5)

                dg = diagp.tile([P, P], bf16, tag="dg")
                nc.vector.tensor_scalar(
                    out=dg[:], in0=eye_sb[:], scalar1=rn[:], scalar2=None,
                    op0=ALU.mult,
                )

                pst = psp.tile([P, D], f32, tag="ps")
                for j in range(KT):
                    # psum[m, u] = rows16[u, j*128+m] * rnorm_u  (transposed+scaled)
                    nc.tensor.matmul(
                        pst[:, j * P : (j + 1) * P],
                        r16[:, j * P : (j + 1) * P],
                        dg[:],
                        start=True,
                        stop=True,
                    )
                # scatter the 8 [128,128] chunks into their k-tiles
                src = pst[:].rearrange("p (k r) -> p k r", k=KT)
                dst = zt[:].rearrange("p (k r) -> p k r", k=KT)[
                    :, :, rt * P : (rt + 1) * P
                ]
                nc.vector.tensor_copy(dst, src)

            # ---------------- Phase B: sim block + exp row-sums -------------
            for m2 in (range(8) if "B" in stages else []):
                for nb in range(8):
                    ps = psp.tile([P, 1024], f32, tag="ps")
                    for k in range(KT):
                        lhsT = zt[:, k * R + m2 * P : k * R + (m2 + 1) * P]
                        for nn in range(2):
                            col = k * R + nb * 1024 + nn * 512
                            nc.tensor.matmul(
                                ps[:, nn * 512 : (nn + 1) * 512],
                                lhsT,
                                zt[:, col : col + 512],
                                start=(k == 0),
                                stop=(k == KT - 1),
                            )
                    idx = m2 * 8 + nb
                    nc.scalar.activation(
                        ps[:], ps[:], FT.Exp, scale=1.0 / TEMP,
                        accum_out=rowsums[:, idx : idx + 1],
                    )

            # ---------------- Phase C: log-denoms + reduction ---------------
            out_sb = statp.tile([1, 2], f32, tag="outsb")
            if "C" not in stages:
                nc.vector.memset(out_sb[:], 0.0)
            if "C" in stages:
                denoms = statp.tile([P, 8], f32, tag="denoms")
                nc.vector.tensor_reduce(
                    denoms[:],
                    rowsums[:].rearrange("p (m n) -> p m n", n=8),
                    axis=mybir.AxisListType.X,
                    op=ALU.add,
                )
                logd = statp.tile([P, 8], f32, tag="logd")
                neg_e2 = statp.tile([P, 1], f32, tag="nege2")
                nc.vector.memset(neg_e2[:], -E2)
                # ln(denom - e^2): masks out the self-similarity term
                nc.scalar.activation(logd[:], denoms[:], FT.Ln, bias=neg_e2[:])

                ps8 = psp.tile([8, 1], f32, tag="ps")
                nc.tensor.matmul(ps8[:], logd[:], ones_f[:], start=True, stop=True)
                sb8 = statp.tile([8, 1], f32, tag="sb8")
                nc.scalar.copy(sb8[:], ps8[:])
                ps1 = psp.tile([1, 1], f32, tag="ps")
                nc.tensor.matmul(ps1[:], sb8[:], ones_f[0:8, :], start=True, stop=True)

                nc.scalar.copy(out_sb[:, 0:1], ps1[:])

            # ---------------- Phase D: positives ----------------------------
            pspos = psp.tile([1, 1024], f32, tag="ps")
            for k in (range(KT) if "D" in stages else []):
                pr = prodp.tile([P, 1024], bf16, tag="pr")
                nc.vector.tensor_tensor(
                    pr[:],
                    zt[:, k * R : k * R + 1024],
                    zt[:, k * R + N : k * R + N + 1024],
                    ALU.mult,
                )
                for h in range(2):
                    nc.tensor.matmul(
                        pspos[:, h * 512 : (h + 1) * 512],
                        ones_b[:],
                        pr[:, h * 512 : (h + 1) * 512],
                        start=(k == 0),
                        stop=(k == KT - 1),
                    )
            if "D" in stages:
                pos_scr = statp.tile([1, 1024], f32, tag="posscr")
                nc.scalar.activation(
                    pos_scr[:], pspos[:], FT.Copy, accum_out=out_sb[:, 1:2]
                )

            nc.sync.dma_start(outd, out_sb[:])

    nc.compile()
    return nc


def _get_nc():
    global _NC
    if _NC is None:
        _NC = _build_nc()
    return _NC


def _in_maps(cat: np.ndarray):
    eye = np.eye(P, dtype=_BF16)
    onesb = np.ones((P, 1), dtype=_BF16)
    onesf = np.ones((P, 1), dtype=np.float32)
    maps = []
    for c in range(8):
        emb_c = np.roll(cat, -BLK * c, axis=0) if c else cat
        maps.append(
            {
                "emb": np.ascontiguousarray(emb_c),
                "eye128": eye,
                "ones_bf16": onesb,
                "ones_f32": onesf,
            }
        )
    return maps


def kernel(emb_i, emb_j):
    emb_i = np.asarray(emb_i, dtype=np.float32)
    emb_j = np.asarray(emb_j, dtype=np.float32)
    assert emb_i.shape == (N, D) and emb_j.shape == (N, D)

    from concourse.bass_utils import run_bass_kernel_spmd

    nc = _get_nc()
    cat = np.concatenate([emb_i, emb_j], axis=0)
    res = run_bass_kernel_spmd(nc, _in_maps(cat), core_ids=list(range(8)))
    logd = sum(float(r["out"][0, 0]) for r in res.results)
    pos = sum(float(r["out"][0, 1]) for r in res.results)
    # sum over all 8 cores covers every positive pair exactly twice == the
    # full 8192-element positives sum.
    loss = (logd - pos / TEMP) / float(R)
    return np.float32(loss)

